# revision 7
# baseline (speedup 1.0000x reference)
"""Trainium2 Bass kernel for ChannelwiseSpatialMHSA.

Math: the reference embeds each scalar pixel x[n,s] as x[n,s]*embed_w, so
Q/K/V rows are scalar multiples of fixed vectors and the whole module
collapses to, per flattened batch n=(b,c) and head h:

    scores[n,h,s,t] = c_h * x[n,s] * x[n,t],   c_h = (q_h . k_h)/sqrt(d_head)
    w[n,h,s] = sum_t softmax_t(scores) * x[n,t]
    out[b,s,o] = sum_h U[h,o] * sum_c merge_w[c] * w[(b,c),h,s]
    U[h,:] = o_w[:, head h] @ v_h   (q/k/v_vec = {q,k,v}_w @ embed_w)

Scores reach +-95, so exp needs exact per-row max subtraction (bias on the
activation), like the reference softmax.

Sharding: 64 sequences (B*C) split 8 per core; weights replicated. Each core
returns a partial [1024, 64] output (its 8 channels merged); host sums the 4
cores belonging to each batch element.
"""

import numpy as np

B, HH, WW, C = 2, 32, 32, 32
S = 1024
D = 64
NH = 4
DH = 16
NCORES = 8
NSEQ = 8  # sequences per core
NSB = 8  # s-blocks of 128 per sequence
GP_SBS = ()  # GpSimd lacks TensorScalarPtr on TRN2 — all reductions on DVE

_CACHE = {}


def _build_nc():
    import concourse.bacc as bacc
    import concourse.tile as tile
    from concourse import mybir

    f32 = mybir.dt.float32
    Alu = mybir.AluOpType
    Act = mybir.ActivationFunctionType

    nc = bacc.Bacc()

    xs = nc.dram_tensor("xs", [NSEQ, S], f32, kind="ExternalInput")
    xe = nc.dram_tensor("xe", [NSEQ, 2], f32, kind="ExternalInput")  # [-max, -min]
    embed_w = nc.dram_tensor("embed_w", [D, 1], f32, kind="ExternalInput")
    q_w = nc.dram_tensor("q_w", [D, D], f32, kind="ExternalInput")
    k_w = nc.dram_tensor("k_w", [D, D], f32, kind="ExternalInput")
    v_w = nc.dram_tensor("v_w", [D, D], f32, kind="ExternalInput")
    o_w = nc.dram_tensor("o_w", [D, D], f32, kind="ExternalInput")
    merge = nc.dram_tensor("merge", [1, NSEQ], f32, kind="ExternalInput")
    hmask = nc.dram_tensor("hmask", [D, NH], f32, kind="ExternalInput")
    ident = nc.dram_tensor("ident", [128, 128], f32, kind="ExternalInput")
    outp = nc.dram_tensor("outp", [S, D], f32, kind="ExternalOutput")
    c_dram = nc.dram_tensor("c_scratch", [1, NH], f32)

    with tile.TileContext(nc) as tc:
        with (
            tc.tile_pool(name="consts", bufs=1) as consts,
            tc.tile_pool(name="seq", bufs=2) as seqp,
            tc.tile_pool(name="et", bufs=4) as etp,
            tc.tile_pool(name="scr", bufs=4) as scrp,
            tc.tile_pool(name="small", bufs=12) as smallp,
            tc.tile_pool(name="ps", bufs=2, space="PSUM") as psp,
            tc.tile_pool(name="mmps", bufs=3, space="PSUM") as mmps,
        ):
            # ---- prologue: fold weights into c[1,4] (per-head score scale)
            # and U[4,64] (per-head output vector) on device ----
            ew_sb = consts.tile([D, 1], f32)
            nc.sync.dma_start(out=ew_sb, in_=embed_w[:, :])
            qT_sb = consts.tile([D, D], f32)
            nc.sync.dma_start(out=qT_sb, in_=q_w.rearrange("o i -> i o"))
            kT_sb = consts.tile([D, D], f32)
            nc.sync.dma_start(out=kT_sb, in_=k_w.rearrange("o i -> i o"))
            vT_sb = consts.tile([D, D], f32)
            nc.sync.dma_start(out=vT_sb, in_=v_w.rearrange("o i -> i o"))
            oT_sb = consts.tile([D, D], f32)
            nc.sync.dma_start(out=oT_sb, in_=o_w.rearrange("o d -> d o"))
            hm_sb = consts.tile([D, NH], f32)
            nc.sync.dma_start(out=hm_sb, in_=hmask[:, :])
            id_sb = consts.tile([128, 128], f32)
            nc.sync.dma_start(out=id_sb, in_=ident[:, :])

            vec_sb = {}
            for name, wT in (("q", qT_sb), ("k", kT_sb), ("v", vT_sb)):
                vps = psp.tile([D, 1], f32, tag="pro")
                nc.tensor.matmul(vps, lhsT=wT, rhs=ew_sb, start=True, stop=True)
                vsb = consts.tile([D, 1], f32, tag=f"{name}vec")
                nc.vector.tensor_copy(vsb, vps)
                vec_sb[name] = vsb

            kvs_sb = consts.tile([D, 1], f32)
            nc.vector.tensor_scalar_mul(kvs_sb, vec_sb["k"], 1.0 / np.sqrt(DH))
            mq_sb = consts.tile([D, NH], f32)
            nc.vector.tensor_scalar_mul(mq_sb, hm_sb, vec_sb["q"])
            mv_sb = consts.tile([D, NH], f32)
            nc.vector.tensor_scalar_mul(mv_sb, hm_sb, vec_sb["v"])

            c_ps = psp.tile([1, NH], f32, tag="pro")
            nc.tensor.matmul(c_ps, lhsT=kvs_sb, rhs=mq_sb, start=True, stop=True)
            c_sb = consts.tile([1, NH], f32)
            nc.vector.tensor_copy(c_sb, c_ps)
            # broadcast c across partitions via a DRAM round-trip
            nc.sync.dma_start(out=c_dram[:, :], in_=c_sb)
            c_bc = consts.tile([128, NH], f32)
            nc.sync.dma_start(out=c_bc, in_=c_dram[0:1, :].to_broadcast([128, NH]))

            u_ps = psp.tile([NH, D], f32, tag="pro")
            nc.tensor.matmul(u_ps, lhsT=mv_sb, rhs=oT_sb, start=True, stop=True)
            u_sb = consts.tile([NH, D], f32)
            nc.vector.tensor_copy(u_sb, u_ps)

            # partial output accumulator, [128, (sb, o)]
            acc_sb = consts.tile([128, NSB, D], f32)
            nc.vector.memset(acc_sb, 0.0)

            xs_cols = xs.rearrange("n (sb p) -> n p sb", p=128)

            for n in range(NSEQ):
                x_bc = seqp.tile([128, S], f32, tag="xbc")
                nc.sync.dma_start(out=x_bc, in_=xs[n : n + 1, :].to_broadcast([128, S]))
                x_col = seqp.tile([128, NSB], f32, tag="xcol")
                nc.sync.dma_start(out=x_col, in_=xs_cols[n])
                nxmax = seqp.tile([128, 1], f32, tag="nxmax")
                nc.sync.dma_start(out=nxmax, in_=xe[n : n + 1, 0:1].to_broadcast([128, 1]))
                nxmin = seqp.tile([128, 1], f32, tag="nxmin")
                nc.sync.dma_start(out=nxmin, in_=xe[n : n + 1, 1:2].to_broadcast([128, 1]))
                merge_col = seqp.tile([NH, 1], f32, tag="mcol")
                nc.sync.dma_start(
                    out=merge_col, in_=merge[0:1, n : n + 1].to_broadcast([NH, 1])
                )
                mu_sb = seqp.tile([NH, D], f32, tag="mu")
                nc.vector.tensor_scalar_mul(mu_sb, u_sb, merge_col)

                w_all = seqp.tile([128, NSB, NH], f32, tag="wall")

                for h in range(NH):
                    # scale_s = c_h * x_s ; bias_s = -max_t scores = min of
                    # scale*(-xmax), scale*(-xmin)
                    scale_m = smallp.tile([128, NSB], f32, tag="scale")
                    nc.vector.tensor_scalar_mul(scale_m, x_col, c_bc[:, h : h + 1])
                    t1 = smallp.tile([128, NSB], f32, tag="t1")
                    nc.vector.tensor_scalar_mul(t1, scale_m, nxmax)
                    t2 = smallp.tile([128, NSB], f32, tag="t2")
                    nc.vector.tensor_scalar_mul(t2, scale_m, nxmin)
                    bias_m = smallp.tile([128, NSB], f32, tag="bias")
                    nc.vector.tensor_tensor(bias_m, t1, t2, op=Alu.min)

                    for sb in range(NSB):
                        den = smallp.tile([128, 1], f32, tag="den")
                        et = etp.tile([128, S], f32, tag="et")
                        nc.scalar.activation(
                            out=et,
                            in_=x_bc,
                            func=Act.Exp,
                            scale=scale_m[:, sb : sb + 1],
                            bias=bias_m[:, sb : sb + 1],
                            accum_out=den,
                        )
                        rec = smallp.tile([128, 1], f32, tag="rec")
                        nc.vector.reciprocal(rec, den)
                        scr = scrp.tile([128, S], f32, tag="scr")
                        eng = nc.gpsimd if sb in GP_SBS else nc.vector
                        eng.scalar_tensor_tensor(
                            out=scr,
                            in0=et,
                            scalar=rec,
                            in1=x_bc,
                            op0=Alu.mult,
                            op1=Alu.mult,
                            accum_out=w_all[:, sb, h : h + 1],
                        )

                # w_all is [128 s, (sb, h)]; transpose to [(sb, h), 128] and
                # contract heads against merge_c * U into acc_sb
                wt_ps = psp.tile([NSB * NH, 128], f32, tag="wT")
                nc.tensor.transpose(wt_ps, w_all[:, :, :], id_sb)
                wt_sb = seqp.tile([NSB * NH, 128], f32, tag="wTsb")
                nc.vector.tensor_copy(wt_sb, wt_ps)
                for sb in range(NSB):
                    lhsT = smallp.tile([NH, 128], f32, tag="lhsT")
                    nc.sync.dma_start(out=lhsT, in_=wt_sb[NH * sb : NH * (sb + 1), :])
                    mm_ps = mmps.tile([128, D], f32, tag="mmout")
                    nc.tensor.matmul(mm_ps, lhsT=lhsT, rhs=mu_sb, start=True, stop=True)
                    nc.vector.tensor_add(acc_sb[:, sb, :], acc_sb[:, sb, :], mm_ps)

            nc.sync.dma_start(
                out=outp.rearrange("(sb p) o -> p sb o", p=128), in_=acc_sb
            )

    if not nc.is_finalized():
        nc.finalize()
    return nc


def kernel(x, embed_w, q_w, k_w, v_w, o_w, merge_w):
    from concourse.bass_utils import run_bass_kernel_spmd

    x = np.asarray(x, dtype=np.float32)
    embed_w = np.asarray(embed_w, dtype=np.float32)
    q_w = np.asarray(q_w, dtype=np.float32)
    k_w = np.asarray(k_w, dtype=np.float32)
    v_w = np.asarray(v_w, dtype=np.float32)
    o_w = np.asarray(o_w, dtype=np.float32)
    merge_w = np.asarray(merge_w, dtype=np.float32)

    if "nc" not in _CACHE:
        _CACHE["nc"] = _build_nc()
    nc = _CACHE["nc"]

    # [B,H,W,C] -> [B*C, S]
    t = np.ascontiguousarray(x.transpose(0, 3, 1, 2).reshape(B * C, S))
    hmask = np.repeat(np.eye(NH, dtype=np.float32), DH, axis=0)  # [64, 4]
    ident = np.eye(128, dtype=np.float32)

    in_maps = []
    for k in range(NCORES):
        sl = np.ascontiguousarray(t[NSEQ * k : NSEQ * (k + 1)])
        xe = np.stack([-sl.max(axis=1), -sl.min(axis=1)], axis=1).astype(np.float32)
        chans = np.arange(NSEQ * k, NSEQ * (k + 1)) % C
        merge = np.ascontiguousarray(merge_w[0, chans].reshape(1, NSEQ))
        in_maps.append(
            dict(
                xs=sl,
                xe=np.ascontiguousarray(xe),
                embed_w=embed_w,
                q_w=q_w,
                k_w=k_w,
                v_w=v_w,
                o_w=o_w,
                merge=merge,
                hmask=hmask,
                ident=ident,
            )
        )

    res = run_bass_kernel_spmd(nc, in_maps, core_ids=list(range(NCORES)))

    out = np.zeros((B, S, D), dtype=np.float32)
    for k in range(NCORES):
        out[k // (NCORES // B)] += res.results[k]["outp"]
    return out.reshape(B, HH, WW, D)
